# revision 13
# baseline (speedup 1.0000x reference)
"""Trainium2 Bass kernel for a masked cross-attention layer.

reference math (fp32):
    Q = emb_cause @ Wq + bq            # [N_C, D]
    K = emb_effect @ Wk + bk           # [N_E, D]
    V = emb_effect @ Wv + bv           # [N_E, D]
    S = (Q @ K.T) / sqrt(D)            # [N_C, N_E]
    S = where(mask == 0, -inf, S)
    A = softmax(S, axis=-1)            # output 2
    O = A @ V                          # output 1

Sharding: rows of emb_cause / mask / outputs split across 8 NeuronCores
(1024 rows each); emb_effect + weights replicated.

Per-core pipeline (memory-bound on 32MB mask in + 32MB attn out):
  - prologue: PE-transpose embeddings, matmul -> QT (scale/bias folded),
    KT (f32), V (bf16, [N_E, D] layout).
  - per 128-row tile: DMA mask; QK matmuls in float32r (full PE rate at
    N=512); ScalarE exp; VectorE scalar_tensor_tensor does mask-multiply
    and row-sum in one pass (softmax without max subtraction -- exact in
    exact arithmetic, values bounded since |S| <~ 3); normalize; DMA attn
    out; PE-transpose P in bf16; PV matmuls accumulate O.
"""

import numpy as np

from concourse import bacc, bass, masks, mybir, tile

N_C, N_E, IN_FEAT, D = 8192, 8192, 256, 128
N_CORES = 8
RPC = N_C // N_CORES          # rows per core = 1024
RT = RPC // 128               # row tiles per core = 8
NE_CHUNKS = N_E // 512        # 16 chunks of 512 for QK / softmax
NE_BLOCKS = N_E // 128        # 64 blocks of 128 for transpose / PV
IN_CHUNKS = IN_FEAT // 128    # 2
SCALE = 1.0 / float(np.sqrt(D))

F32 = mybir.dt.float32
F32R = mybir.dt.float32r
BF16 = mybir.dt.bfloat16
I32 = mybir.dt.int32

AF = mybir.ActivationFunctionType
ALU = mybir.AluOpType
ts = bass.ts


def _r(ap):
    """View a float32 AP as float32r for full-rate PE matmuls."""
    return ap.bitcast(F32R)


def build(main_reps=1):
    nc = bacc.Bacc("TRN2", target_bir_lowering=False, debug=False)

    emb_cause = nc.dram_tensor("emb_cause", [RPC, IN_FEAT], F32, kind="ExternalInput")
    emb_effect = nc.dram_tensor("emb_effect", [N_E, IN_FEAT], F32, kind="ExternalInput")
    causal_mask = nc.dram_tensor("causal_mask", [RPC, N_E], I32, kind="ExternalInput")
    wq_d = nc.dram_tensor("Wq", [IN_FEAT, D], F32, kind="ExternalInput")
    wk_d = nc.dram_tensor("Wk", [IN_FEAT, D], F32, kind="ExternalInput")
    wv_d = nc.dram_tensor("Wv", [IN_FEAT, D], F32, kind="ExternalInput")
    bq_d = nc.dram_tensor("bq", [D, 1], F32, kind="ExternalInput")
    bk_d = nc.dram_tensor("bk", [D, 1], F32, kind="ExternalInput")
    bv_d = nc.dram_tensor("bv", [D, 1], F32, kind="ExternalInput")
    attn_d = nc.dram_tensor("attn", [RPC, N_E], F32, kind="ExternalOutput")
    causal_d = nc.dram_tensor("causal", [RPC, D], F32, kind="ExternalOutput")

    with tile.TileContext(nc) as tc:
        with (
            tc.tile_pool(name="persist", bufs=1) as persist,
            tc.tile_pool(name="consts", bufs=1) as consts,
        ):
            ident = consts.tile([128, 128], F32)
            masks.make_identity(nc, ident[:])
            ident_bf = consts.tile([128, 128], BF16)
            masks.make_identity(nc, ident_bf[:])

            # weights: [:, c*128:(c+1)*128] holds rows c*128.. of W (lhsT chunks)
            wq_raw = persist.tile([128, IN_FEAT], F32)
            wk_raw = persist.tile([128, IN_FEAT], F32)
            wv_raw = persist.tile([128, IN_FEAT], F32)
            wq = persist.tile([128, IN_FEAT], F32R)
            wk = persist.tile([128, IN_FEAT], F32R)
            wv = persist.tile([128, IN_FEAT], F32R)
            for c in range(IN_CHUNKS):
                nc.sync.dma_start(out=wq_raw[:, ts(c, 128)], in_=wq_d[ts(c, 128), :])
                nc.sync.dma_start(out=wk_raw[:, ts(c, 128)], in_=wk_d[ts(c, 128), :])
                nc.sync.dma_start(out=wv_raw[:, ts(c, 128)], in_=wv_d[ts(c, 128), :])
            nc.vector.tensor_copy(wq[:], wq_raw[:])
            nc.vector.tensor_copy(wk[:], wk_raw[:])
            nc.vector.tensor_copy(wv[:], wv_raw[:])
            bq = persist.tile([128, 1], F32)
            bk = persist.tile([128, 1], F32)
            bv = persist.tile([128, 1], F32)
            nc.sync.dma_start(out=bq[:], in_=bq_d[:])
            nc.sync.dma_start(out=bk[:], in_=bk_d[:])
            nc.sync.dma_start(out=bv[:], in_=bv_d[:])
            bqs = persist.tile([128, 1], F32)  # bq * SCALE
            nc.scalar.mul(bqs[:], bq[:], SCALE)

            # persistent big operands
            kt = persist.tile([128, N_E], F32R)        # K^T  [D, N_E]
            qt = persist.tile([128, RPC], F32R)        # Q^T  [D, RPC] (scale folded)
            v_sb = persist.tile([128, NE_BLOCKS, D], BF16)  # V [N_E, D] blocks

            # ---------------- prologue ----------------
            with (
                tc.tile_pool(name="pro_sb", bufs=4) as pro_sb,
                tc.tile_pool(name="pro_eeT", bufs=1) as pro_eeT,
                tc.tile_pool(name="pro_ecT", bufs=1) as pro_ecT,
                tc.tile_pool(name="pro_ps", bufs=4, space="PSUM") as pro_ps,
                tc.tile_pool(name="pro_ps2", bufs=2, space="PSUM") as pro_ps2,
                tc.tile_pool(name="pro_vt", bufs=2) as pro_vt,
                tc.tile_pool(name="pro_vps", bufs=2, space="PSUM") as pro_vps,
            ):
                eeT = pro_eeT.tile([128, IN_CHUNKS, N_E], F32R)  # emb_effect^T
                ecT = pro_ecT.tile([128, IN_CHUNKS, RPC], F32R)  # emb_cause^T

                def load_transposed(src, dst, r4):
                    raws = []
                    for rr in range(4):
                        raw = pro_sb.tile([128, IN_FEAT], F32, name="raw")
                        nc.sync.dma_start(out=raw[:], in_=src[ts(4 * r4 + rr, 128), :])
                        raws.append(raw)
                    for c in range(IN_CHUNKS):
                        tp = pro_ps.tile([128, 512], F32, name="tp")
                        for rr in range(4):
                            nc.tensor.transpose(
                                tp[:, ts(rr, 128)], raws[rr][:, ts(c, 128)], ident[:]
                            )
                        nc.vector.tensor_copy(dst[:, c, ts(r4, 512)], tp[:])

                # emb_cause first: QT unblocks the first QK matmuls
                for j in range(RPC // 512):
                    load_transposed(emb_cause, ecT, j)
                    ps = pro_ps2.tile([128, 512], F32, name="ps")
                    for c in range(IN_CHUNKS):
                        nc.tensor.matmul(
                            ps[:], wq[:, ts(c, 128)], ecT[:, c, ts(j, 512)],
                            start=(c == 0), stop=(c == IN_CHUNKS - 1),
                        )
                    nc.scalar.activation(
                        qt[:, ts(j, 512)], ps[:], AF.Identity, bias=bqs[:], scale=SCALE
                    )

                # emb_effect per 512-row group: KT chunk then V chunk
                for j in range(NE_CHUNKS):
                    load_transposed(emb_effect, eeT, j)
                    ps = pro_ps2.tile([128, 512], F32, name="ps")
                    for c in range(IN_CHUNKS):
                        nc.tensor.matmul(
                            ps[:], wk[:, ts(c, 128)], eeT[:, c, ts(j, 512)],
                            start=(c == 0), stop=(c == IN_CHUNKS - 1),
                        )
                    nc.scalar.activation(kt[:, ts(j, 512)], ps[:], AF.Identity, bias=bk[:])

                    ps = pro_ps2.tile([128, 512], F32, name="ps")
                    for c in range(IN_CHUNKS):
                        nc.tensor.matmul(
                            ps[:], wv[:, ts(c, 128)], eeT[:, c, ts(j, 512)],
                            start=(c == 0), stop=(c == IN_CHUNKS - 1),
                        )
                    vt = pro_vt.tile([128, 512], BF16, name="vt")
                    nc.scalar.activation(vt[:], ps[:], AF.Identity, bias=bv[:])
                    vps = pro_vps.tile([128, 512], BF16, name="vps")
                    for b in range(4):
                        nc.tensor.transpose(
                            vps[:, ts(b, 128)], vt[:, ts(b, 128)], ident_bf[:]
                        )
                    nc.vector.tensor_copy(v_sb[:, 4 * j : 4 * j + 4, :], vps[:])

            # ---------------- main loop over row tiles ----------------
            with (
                tc.tile_pool(name="mask_p", bufs=2) as mask_p,
                tc.tile_pool(name="em_p", bufs=2) as em_p,
                tc.tile_pool(name="stat_p", bufs=2) as stat_p,
                tc.tile_pool(name="pt_p", bufs=4) as pt_p,
                tc.tile_pool(name="co_p", bufs=2) as co_p,
                tc.tile_pool(name="qk_ps", bufs=3, space="PSUM") as qk_ps,
                tc.tile_pool(name="tp_ps", bufs=3, space="PSUM") as tp_ps,
                tc.tile_pool(name="pv_ps", bufs=2, space="PSUM") as pv_ps,
            ):
                for rt in [r for _ in range(main_reps) for r in range(RT)]:
                    mask_t = mask_p.tile([128, N_E], I32, name="mask_t")
                    nc.sync.dma_start(out=mask_t[:], in_=causal_mask[ts(rt, 128), :])

                    em = em_p.tile([128, N_E], F32, name="em")
                    for j in range(NE_CHUNKS):
                        ps = qk_ps.tile([128, 512], F32, name="ps")
                        nc.tensor.matmul(
                            ps[:], qt[:, ts(rt, 128)], kt[:, ts(j, 512)],
                            start=True, stop=True,
                        )
                        nc.scalar.activation(em[:, ts(j, 512)], ps[:], AF.Exp)

                    # em := mask * em (unnormalized P), rowsum comes free
                    rs = stat_p.tile([128, 1], F32, name="rs")
                    nc.vector.scalar_tensor_tensor(
                        out=em[:],
                        in0=mask_t[:],
                        scalar=1.0,
                        in1=em[:],
                        op0=ALU.mult,
                        op1=ALU.mult,
                        accum_out=rs[:],
                    )
                    recip = stat_p.tile([128, 1], F32, name="recip")
                    nc.vector.reciprocal(recip[:], rs[:])

                    # attn = em * recip, written into the dead mask buffer;
                    # chunked so stores start before the whole row is normalized
                    attn_t = mask_t.bitcast(F32)
                    for q in range(4):
                        nc.scalar.mul(
                            attn_t[:, ts(q, 2048)], em[:, ts(q, 2048)], recip[:]
                        )
                        nc.scalar.dma_start(
                            out=attn_d[ts(rt, 128), ts(q, 2048)],
                            in_=attn_t[:, ts(q, 2048)],
                        )

                    # transpose unnormalized P (f32 -> bf16 on batched copy-out),
                    # accumulate O' = P @ V ; O = O' * recip on the way out
                    co_ps = pv_ps.tile([128, D], F32, name="co_ps")
                    for g in range(NE_BLOCKS // 4):
                        tp = tp_ps.tile([128, 512], F32, name="tp")
                        for b in range(4):
                            blk = 4 * g + b
                            nc.tensor.transpose(
                                tp[:, ts(b, 128)], em[:, ts(blk, 128)], ident[:]
                            )
                        pt = pt_p.tile([128, 512], BF16, name="pt")
                        nc.vector.tensor_copy(pt[:], tp[:])
                        for b in range(4):
                            blk = 4 * g + b
                            nc.tensor.matmul(
                                co_ps[:], pt[:, ts(b, 128)], v_sb[:, blk, :],
                                start=(blk == 0), stop=(blk == NE_BLOCKS - 1),
                            )

                    co = co_p.tile([128, D], F32, name="co")
                    nc.scalar.mul(co[:], co_ps[:], recip[:])
                    nc.scalar.dma_start(out=causal_d[ts(rt, 128), :], in_=co[:])

    nc.compile()
    return nc


_NC_CACHE = {}


def _get_nc():
    if "nc" not in _NC_CACHE:
        _NC_CACHE["nc"] = build()
    return _NC_CACHE["nc"]


def kernel(emb_cause, emb_effect, causal_mask, Wq, bq, Wk, bk, Wv, bv):
    from concourse.bass_utils import run_bass_kernel_spmd

    emb_cause = np.ascontiguousarray(np.asarray(emb_cause, dtype=np.float32))
    emb_effect = np.ascontiguousarray(np.asarray(emb_effect, dtype=np.float32))
    causal_mask = np.ascontiguousarray(np.asarray(causal_mask, dtype=np.int32))
    shared = {
        "emb_effect": emb_effect,
        "Wq": np.ascontiguousarray(np.asarray(Wq, dtype=np.float32)),
        "Wk": np.ascontiguousarray(np.asarray(Wk, dtype=np.float32)),
        "Wv": np.ascontiguousarray(np.asarray(Wv, dtype=np.float32)),
        "bq": np.ascontiguousarray(np.asarray(bq, dtype=np.float32).reshape(D, 1)),
        "bk": np.ascontiguousarray(np.asarray(bk, dtype=np.float32).reshape(D, 1)),
        "bv": np.ascontiguousarray(np.asarray(bv, dtype=np.float32).reshape(D, 1)),
    }
    in_maps = []
    for c in range(N_CORES):
        rows = slice(c * RPC, (c + 1) * RPC)
        in_maps.append(
            {
                "emb_cause": np.ascontiguousarray(emb_cause[rows]),
                "causal_mask": np.ascontiguousarray(causal_mask[rows]),
                **shared,
            }
        )

    nc = _get_nc()
    res = run_bass_kernel_spmd(nc, in_maps, core_ids=list(range(N_CORES)))
    causal = np.concatenate([r["causal"] for r in res.results], axis=0)
    attn = np.concatenate([r["attn"] for r in res.results], axis=0)
    return causal, attn


# revision 14
# speedup vs baseline: 1.2736x; 1.2736x over previous
"""Trainium2 Bass kernel for a masked cross-attention layer.

reference math (fp32):
    Q = emb_cause @ Wq + bq            # [N_C, D]
    K = emb_effect @ Wk + bk           # [N_E, D]
    V = emb_effect @ Wv + bv           # [N_E, D]
    S = (Q @ K.T) / sqrt(D)            # [N_C, N_E]
    S = where(mask == 0, -inf, S)
    A = softmax(S, axis=-1)            # output 2
    O = A @ V                          # output 1

Sharding: rows of emb_cause / mask / outputs split across 8 NeuronCores
(1024 rows each); emb_effect + weights replicated.

Per-core pipeline (memory-bound on 32MB mask in + 32MB attn out):
  - prologue: PE-transpose embeddings, matmul -> QT (scale/bias folded),
    KT (f32), V (bf16, [N_E, D] layout).
  - per 128-row tile: DMA mask; QK matmuls in float32r (full PE rate at
    N=512); ScalarE exp; VectorE scalar_tensor_tensor does mask-multiply
    and row-sum in one pass (softmax without max subtraction -- exact in
    exact arithmetic, values bounded since |S| <~ 3); normalize; DMA attn
    out; PE-transpose P in bf16; PV matmuls accumulate O.
"""

import numpy as np

from concourse import bacc, bass, masks, mybir, tile

N_C, N_E, IN_FEAT, D = 8192, 8192, 256, 128
N_CORES = 8
RPC = N_C // N_CORES          # rows per core = 1024
RT = RPC // 128               # row tiles per core = 8
NE_CHUNKS = N_E // 512        # 16 chunks of 512 for QK / softmax
NE_BLOCKS = N_E // 128        # 64 blocks of 128 for transpose / PV
IN_CHUNKS = IN_FEAT // 128    # 2
SCALE = 1.0 / float(np.sqrt(D))

F32 = mybir.dt.float32
F32R = mybir.dt.float32r
BF16 = mybir.dt.bfloat16
I32 = mybir.dt.int32

AF = mybir.ActivationFunctionType
ALU = mybir.AluOpType
ts = bass.ts


def _r(ap):
    """View a float32 AP as float32r for full-rate PE matmuls."""
    return ap.bitcast(F32R)


def build(main_reps=1):
    nc = bacc.Bacc("TRN2", target_bir_lowering=False, debug=False)

    emb_cause = nc.dram_tensor("emb_cause", [RPC, IN_FEAT], F32, kind="ExternalInput")
    emb_effect = nc.dram_tensor("emb_effect", [N_E, IN_FEAT], F32, kind="ExternalInput")
    causal_mask = nc.dram_tensor("causal_mask", [RPC, N_E], I32, kind="ExternalInput")
    wq_d = nc.dram_tensor("Wq", [IN_FEAT, D], F32, kind="ExternalInput")
    wk_d = nc.dram_tensor("Wk", [IN_FEAT, D], F32, kind="ExternalInput")
    wv_d = nc.dram_tensor("Wv", [IN_FEAT, D], F32, kind="ExternalInput")
    bq_d = nc.dram_tensor("bq", [D, 1], F32, kind="ExternalInput")
    bk_d = nc.dram_tensor("bk", [D, 1], F32, kind="ExternalInput")
    bv_d = nc.dram_tensor("bv", [D, 1], F32, kind="ExternalInput")
    attn_d = nc.dram_tensor("attn", [RPC, N_E], F32, kind="ExternalOutput")
    causal_d = nc.dram_tensor("causal", [RPC, D], F32, kind="ExternalOutput")

    with tile.TileContext(nc) as tc:
        with (
            tc.tile_pool(name="persist", bufs=1) as persist,
            tc.tile_pool(name="consts", bufs=1) as consts,
        ):
            ident = consts.tile([128, 128], F32)
            masks.make_identity(nc, ident[:])
            ident_bf = consts.tile([128, 128], BF16)
            masks.make_identity(nc, ident_bf[:])

            # weights: [:, c*128:(c+1)*128] holds rows c*128.. of W (lhsT chunks)
            wq_raw = persist.tile([128, IN_FEAT], F32)
            wk_raw = persist.tile([128, IN_FEAT], F32)
            wv_raw = persist.tile([128, IN_FEAT], F32)
            wq = persist.tile([128, IN_FEAT], F32R)
            wk = persist.tile([128, IN_FEAT], F32R)
            wv = persist.tile([128, IN_FEAT], F32R)
            for c in range(IN_CHUNKS):
                nc.sync.dma_start(out=wq_raw[:, ts(c, 128)], in_=wq_d[ts(c, 128), :])
                nc.sync.dma_start(out=wk_raw[:, ts(c, 128)], in_=wk_d[ts(c, 128), :])
                nc.sync.dma_start(out=wv_raw[:, ts(c, 128)], in_=wv_d[ts(c, 128), :])
            nc.vector.tensor_copy(wq[:], wq_raw[:])
            nc.vector.tensor_copy(wk[:], wk_raw[:])
            nc.vector.tensor_copy(wv[:], wv_raw[:])
            bq = persist.tile([128, 1], F32)
            bk = persist.tile([128, 1], F32)
            bv = persist.tile([128, 1], F32)
            nc.sync.dma_start(out=bq[:], in_=bq_d[:])
            nc.sync.dma_start(out=bk[:], in_=bk_d[:])
            nc.sync.dma_start(out=bv[:], in_=bv_d[:])
            bqs = persist.tile([128, 1], F32)  # bq * SCALE
            nc.scalar.mul(bqs[:], bq[:], SCALE)

            # persistent big operands
            kt = persist.tile([128, N_E], F32R)        # K^T  [D, N_E]
            qt = persist.tile([128, RPC], F32R)        # Q^T  [D, RPC] (scale folded)
            v_sb = persist.tile([128, NE_BLOCKS, D], BF16)  # V [N_E, D] blocks

            # ---------------- prologue ----------------
            with (
                tc.tile_pool(name="pro_sb", bufs=4) as pro_sb,
                tc.tile_pool(name="pro_eeT", bufs=1) as pro_eeT,
                tc.tile_pool(name="pro_ecT", bufs=1) as pro_ecT,
                tc.tile_pool(name="pro_ps", bufs=4, space="PSUM") as pro_ps,
                tc.tile_pool(name="pro_ps2", bufs=2, space="PSUM") as pro_ps2,
                tc.tile_pool(name="pro_vt", bufs=2) as pro_vt,
                tc.tile_pool(name="pro_vps", bufs=2, space="PSUM") as pro_vps,
            ):
                eeT = pro_eeT.tile([128, IN_CHUNKS, N_E], F32R)  # emb_effect^T
                ecT = pro_ecT.tile([128, IN_CHUNKS, RPC], F32R)  # emb_cause^T

                def load_transposed(src, dst, r4):
                    raws = []
                    for rr in range(4):
                        raw = pro_sb.tile([128, IN_FEAT], F32, name="raw")
                        nc.sync.dma_start(out=raw[:], in_=src[ts(4 * r4 + rr, 128), :])
                        raws.append(raw)
                    for c in range(IN_CHUNKS):
                        tp = pro_ps.tile([128, 512], F32, name="tp")
                        for rr in range(4):
                            nc.tensor.transpose(
                                tp[:, ts(rr, 128)], raws[rr][:, ts(c, 128)], ident[:]
                            )
                        nc.vector.tensor_copy(dst[:, c, ts(r4, 512)], tp[:])

                # emb_cause first: QT unblocks the first QK matmuls
                for j in range(RPC // 512):
                    load_transposed(emb_cause, ecT, j)
                    ps = pro_ps2.tile([128, 512], F32, name="ps")
                    for c in range(IN_CHUNKS):
                        nc.tensor.matmul(
                            ps[:], wq[:, ts(c, 128)], ecT[:, c, ts(j, 512)],
                            start=(c == 0), stop=(c == IN_CHUNKS - 1),
                        )
                    nc.scalar.activation(
                        qt[:, ts(j, 512)], ps[:], AF.Identity, bias=bqs[:], scale=SCALE
                    )

                # emb_effect per 512-row group: KT chunk then V chunk
                for j in range(NE_CHUNKS):
                    load_transposed(emb_effect, eeT, j)
                    ps = pro_ps2.tile([128, 512], F32, name="ps")
                    for c in range(IN_CHUNKS):
                        nc.tensor.matmul(
                            ps[:], wk[:, ts(c, 128)], eeT[:, c, ts(j, 512)],
                            start=(c == 0), stop=(c == IN_CHUNKS - 1),
                        )
                    nc.scalar.activation(kt[:, ts(j, 512)], ps[:], AF.Identity, bias=bk[:])

                    ps = pro_ps2.tile([128, 512], F32, name="ps")
                    for c in range(IN_CHUNKS):
                        nc.tensor.matmul(
                            ps[:], wv[:, ts(c, 128)], eeT[:, c, ts(j, 512)],
                            start=(c == 0), stop=(c == IN_CHUNKS - 1),
                        )
                    vt = pro_vt.tile([128, 512], BF16, name="vt")
                    nc.scalar.activation(vt[:], ps[:], AF.Identity, bias=bv[:])
                    vps = pro_vps.tile([128, 512], BF16, name="vps")
                    for b in range(4):
                        nc.tensor.transpose(
                            vps[:, ts(b, 128)], vt[:, ts(b, 128)], ident_bf[:]
                        )
                    nc.vector.tensor_copy(v_sb[:, 4 * j : 4 * j + 4, :], vps[:])

            # ---------------- main loop over row tiles ----------------
            with (
                tc.tile_pool(name="mask_p", bufs=2) as mask_p,
                tc.tile_pool(name="em_p", bufs=2) as em_p,
                tc.tile_pool(name="stat_p", bufs=2) as stat_p,
                tc.tile_pool(name="pt_p", bufs=4) as pt_p,
                tc.tile_pool(name="co_p", bufs=2) as co_p,
                tc.tile_pool(name="qk_ps", bufs=3, space="PSUM") as qk_ps,
                tc.tile_pool(name="tp_ps", bufs=2, space="PSUM") as tp_ps,
                tc.tile_pool(name="pv_ps", bufs=2, space="PSUM") as pv_ps,
            ):
                for rt in [r for _ in range(main_reps) for r in range(RT)]:
                    mask_t = mask_p.tile([128, N_E], I32, name="mask_t")
                    nc.sync.dma_start(out=mask_t[:], in_=causal_mask[ts(rt, 128), :])

                    em = em_p.tile([128, N_E], F32, name="em")
                    for j in range(NE_CHUNKS):
                        ps = qk_ps.tile([128, 512], F32, name="ps")
                        nc.tensor.matmul(
                            ps[:], qt[:, ts(rt, 128)], kt[:, ts(j, 512)],
                            start=True, stop=True,
                        )
                        nc.scalar.activation(em[:, ts(j, 512)], ps[:], AF.Exp)

                    # em := mask * em (unnormalized P), rowsum comes free
                    rs = stat_p.tile([128, 1], F32, name="rs")
                    nc.vector.scalar_tensor_tensor(
                        out=em[:],
                        in0=mask_t[:],
                        scalar=1.0,
                        in1=em[:],
                        op0=ALU.mult,
                        op1=ALU.mult,
                        accum_out=rs[:],
                    )
                    recip = stat_p.tile([128, 1], F32, name="recip")
                    nc.vector.reciprocal(recip[:], rs[:])

                    # attn = em * recip, written into the dead mask buffer
                    attn_t = mask_t.bitcast(F32)
                    nc.scalar.mul(attn_t[:], em[:], recip[:])
                    nc.scalar.dma_start(out=attn_d[ts(rt, 128), :], in_=attn_t[:])

                    # transpose unnormalized P (f32 -> bf16 on batched copy-out),
                    # accumulate O' = P @ V ; O = O' * recip on the way out
                    co_ps = pv_ps.tile([128, D], F32, name="co_ps")
                    for g in range(NE_BLOCKS // 4):
                        tp = tp_ps.tile([128, 512], F32, name="tp")
                        for b in range(4):
                            blk = 4 * g + b
                            nc.tensor.transpose(
                                tp[:, ts(b, 128)], em[:, ts(blk, 128)], ident[:]
                            )
                        pt = pt_p.tile([128, 512], BF16, name="pt")
                        nc.vector.tensor_copy(pt[:], tp[:])
                        for b in range(4):
                            blk = 4 * g + b
                            nc.tensor.matmul(
                                co_ps[:], pt[:, ts(b, 128)], v_sb[:, blk, :],
                                start=(blk == 0), stop=(blk == NE_BLOCKS - 1),
                            )

                    co = co_p.tile([128, D], F32, name="co")
                    nc.scalar.mul(co[:], co_ps[:], recip[:])
                    nc.scalar.dma_start(out=causal_d[ts(rt, 128), :], in_=co[:])

    nc.compile()
    return nc


_NC_CACHE = {}


def _get_nc():
    if "nc" not in _NC_CACHE:
        _NC_CACHE["nc"] = build()
    return _NC_CACHE["nc"]


def kernel(emb_cause, emb_effect, causal_mask, Wq, bq, Wk, bk, Wv, bv):
    from concourse.bass_utils import run_bass_kernel_spmd

    emb_cause = np.ascontiguousarray(np.asarray(emb_cause, dtype=np.float32))
    emb_effect = np.ascontiguousarray(np.asarray(emb_effect, dtype=np.float32))
    causal_mask = np.ascontiguousarray(np.asarray(causal_mask, dtype=np.int32))
    shared = {
        "emb_effect": emb_effect,
        "Wq": np.ascontiguousarray(np.asarray(Wq, dtype=np.float32)),
        "Wk": np.ascontiguousarray(np.asarray(Wk, dtype=np.float32)),
        "Wv": np.ascontiguousarray(np.asarray(Wv, dtype=np.float32)),
        "bq": np.ascontiguousarray(np.asarray(bq, dtype=np.float32).reshape(D, 1)),
        "bk": np.ascontiguousarray(np.asarray(bk, dtype=np.float32).reshape(D, 1)),
        "bv": np.ascontiguousarray(np.asarray(bv, dtype=np.float32).reshape(D, 1)),
    }
    in_maps = []
    for c in range(N_CORES):
        rows = slice(c * RPC, (c + 1) * RPC)
        in_maps.append(
            {
                "emb_cause": np.ascontiguousarray(emb_cause[rows]),
                "causal_mask": np.ascontiguousarray(causal_mask[rows]),
                **shared,
            }
        )

    nc = _get_nc()
    res = run_bass_kernel_spmd(nc, in_maps, core_ids=list(range(N_CORES)))
    causal = np.concatenate([r["causal"] for r in res.results], axis=0)
    attn = np.concatenate([r["attn"] for r in res.results], axis=0)
    return causal, attn


# revision 15
# speedup vs baseline: 1.4436x; 1.1335x over previous
"""Trainium2 Bass kernel for a masked cross-attention layer.

reference math (fp32):
    Q = emb_cause @ Wq + bq            # [N_C, D]
    K = emb_effect @ Wk + bk           # [N_E, D]
    V = emb_effect @ Wv + bv           # [N_E, D]
    S = (Q @ K.T) / sqrt(D)            # [N_C, N_E]
    S = where(mask == 0, -inf, S)
    A = softmax(S, axis=-1)            # output 2
    O = A @ V                          # output 1

Sharding: rows of emb_cause / mask / outputs split across 8 NeuronCores
(1024 rows each); emb_effect + weights replicated.

Per-core pipeline (memory-bound on 32MB mask in + 32MB attn out):
  - prologue: PE-transpose embeddings, matmul -> QT (scale/bias folded),
    KT (f32), V (bf16, [N_E, D] layout).
  - per 128-row tile: DMA mask; QK matmuls in float32r (full PE rate at
    N=512); ScalarE exp; VectorE scalar_tensor_tensor does mask-multiply
    and row-sum in one pass (softmax without max subtraction -- exact in
    exact arithmetic, values bounded since |S| <~ 3); normalize; DMA attn
    out; PE-transpose P in bf16; PV matmuls accumulate O.
"""

import numpy as np

from concourse import bacc, bass, masks, mybir, tile

N_C, N_E, IN_FEAT, D = 8192, 8192, 256, 128
N_CORES = 8
RPC = N_C // N_CORES          # rows per core = 1024
RT = RPC // 128               # row tiles per core = 8
NE_CHUNKS = N_E // 512        # 16 chunks of 512 for QK / softmax
NE_BLOCKS = N_E // 128        # 64 blocks of 128 for transpose / PV
IN_CHUNKS = IN_FEAT // 128    # 2
SCALE = 1.0 / float(np.sqrt(D))

F32 = mybir.dt.float32
F32R = mybir.dt.float32r
BF16 = mybir.dt.bfloat16
I32 = mybir.dt.int32

AF = mybir.ActivationFunctionType
ALU = mybir.AluOpType
ts = bass.ts


def _r(ap):
    """View a float32 AP as float32r for full-rate PE matmuls."""
    return ap.bitcast(F32R)


def build(main_reps=1):
    nc = bacc.Bacc("TRN2", target_bir_lowering=False, debug=False)

    emb_cause = nc.dram_tensor("emb_cause", [RPC, IN_FEAT], F32, kind="ExternalInput")
    emb_effect = nc.dram_tensor("emb_effect", [N_E, IN_FEAT], F32, kind="ExternalInput")
    causal_mask = nc.dram_tensor("causal_mask", [RPC, N_E], I32, kind="ExternalInput")
    wq_d = nc.dram_tensor("Wq", [IN_FEAT, D], F32, kind="ExternalInput")
    wk_d = nc.dram_tensor("Wk", [IN_FEAT, D], F32, kind="ExternalInput")
    wv_d = nc.dram_tensor("Wv", [IN_FEAT, D], F32, kind="ExternalInput")
    bq_d = nc.dram_tensor("bq", [D, 1], F32, kind="ExternalInput")
    bk_d = nc.dram_tensor("bk", [D, 1], F32, kind="ExternalInput")
    bv_d = nc.dram_tensor("bv", [D, 1], F32, kind="ExternalInput")
    attn_d = nc.dram_tensor("attn", [RPC, N_E], F32, kind="ExternalOutput")
    causal_d = nc.dram_tensor("causal", [RPC, D], F32, kind="ExternalOutput")

    with tile.TileContext(nc) as tc:
        with (
            tc.tile_pool(name="persist", bufs=1) as persist,
            tc.tile_pool(name="consts", bufs=1) as consts,
        ):
            ident = consts.tile([128, 128], F32)
            masks.make_identity(nc, ident[:])
            ident_bf = consts.tile([128, 128], BF16)
            masks.make_identity(nc, ident_bf[:])

            # weights: [:, c*128:(c+1)*128] holds rows c*128.. of W (lhsT chunks)
            wq_raw = persist.tile([128, IN_FEAT], F32)
            wk_raw = persist.tile([128, IN_FEAT], F32)
            wv_raw = persist.tile([128, IN_FEAT], F32)
            wq = persist.tile([128, IN_FEAT], F32R)
            wk = persist.tile([128, IN_FEAT], F32R)
            wv = persist.tile([128, IN_FEAT], F32R)
            for c in range(IN_CHUNKS):
                nc.sync.dma_start(out=wq_raw[:, ts(c, 128)], in_=wq_d[ts(c, 128), :])
                nc.sync.dma_start(out=wk_raw[:, ts(c, 128)], in_=wk_d[ts(c, 128), :])
                nc.sync.dma_start(out=wv_raw[:, ts(c, 128)], in_=wv_d[ts(c, 128), :])
            nc.vector.tensor_copy(wq[:], wq_raw[:])
            nc.vector.tensor_copy(wk[:], wk_raw[:])
            nc.vector.tensor_copy(wv[:], wv_raw[:])
            bq = persist.tile([128, 1], F32)
            bk = persist.tile([128, 1], F32)
            bv = persist.tile([128, 1], F32)
            nc.sync.dma_start(out=bq[:], in_=bq_d[:])
            nc.sync.dma_start(out=bk[:], in_=bk_d[:])
            nc.sync.dma_start(out=bv[:], in_=bv_d[:])
            bqs = persist.tile([128, 1], F32)  # bq * SCALE
            nc.scalar.mul(bqs[:], bq[:], SCALE)

            # persistent big operands
            kt = persist.tile([128, N_E], F32R)        # K^T  [D, N_E]
            qt = persist.tile([128, RPC], F32R)        # Q^T  [D, RPC] (scale folded)
            v_sb = persist.tile([128, NE_BLOCKS, D], BF16)  # V [N_E, D] blocks

            # ---------------- prologue ----------------
            with (
                tc.tile_pool(name="pro_sb", bufs=4) as pro_sb,
                tc.tile_pool(name="pro_eeT", bufs=1) as pro_eeT,
                tc.tile_pool(name="pro_ecT", bufs=1) as pro_ecT,
                tc.tile_pool(name="pro_ps", bufs=4, space="PSUM") as pro_ps,
                tc.tile_pool(name="pro_ps2", bufs=2, space="PSUM") as pro_ps2,
                tc.tile_pool(name="pro_vt", bufs=2) as pro_vt,
                tc.tile_pool(name="pro_vps", bufs=2, space="PSUM") as pro_vps,
            ):
                eeT = pro_eeT.tile([128, IN_CHUNKS, N_E], F32R)  # emb_effect^T
                ecT = pro_ecT.tile([128, IN_CHUNKS, RPC], F32R)  # emb_cause^T

                def load_transposed(src, dst, r4):
                    raws = []
                    for rr in range(4):
                        raw = pro_sb.tile([128, IN_FEAT], F32, name="raw")
                        nc.sync.dma_start(out=raw[:], in_=src[ts(4 * r4 + rr, 128), :])
                        raws.append(raw)
                    for c in range(IN_CHUNKS):
                        tp = pro_ps.tile([128, 512], F32, name="tp")
                        for rr in range(4):
                            nc.tensor.transpose(
                                tp[:, ts(rr, 128)], raws[rr][:, ts(c, 128)], ident[:]
                            )
                        nc.vector.tensor_copy(dst[:, c, ts(r4, 512)], tp[:])

                # emb_cause first: QT unblocks the first QK matmuls
                for j in range(RPC // 512):
                    load_transposed(emb_cause, ecT, j)
                    ps = pro_ps2.tile([128, 512], F32, name="ps")
                    for c in range(IN_CHUNKS):
                        nc.tensor.matmul(
                            ps[:], wq[:, ts(c, 128)], ecT[:, c, ts(j, 512)],
                            start=(c == 0), stop=(c == IN_CHUNKS - 1),
                        )
                    nc.scalar.activation(
                        qt[:, ts(j, 512)], ps[:], AF.Identity, bias=bqs[:], scale=SCALE
                    )

                # emb_effect per 512-row group: KT chunk then V chunk
                for j in range(NE_CHUNKS):
                    load_transposed(emb_effect, eeT, j)
                    ps = pro_ps2.tile([128, 512], F32, name="ps")
                    for c in range(IN_CHUNKS):
                        nc.tensor.matmul(
                            ps[:], wk[:, ts(c, 128)], eeT[:, c, ts(j, 512)],
                            start=(c == 0), stop=(c == IN_CHUNKS - 1),
                        )
                    nc.scalar.activation(kt[:, ts(j, 512)], ps[:], AF.Identity, bias=bk[:])

                    ps = pro_ps2.tile([128, 512], F32, name="ps")
                    for c in range(IN_CHUNKS):
                        nc.tensor.matmul(
                            ps[:], wv[:, ts(c, 128)], eeT[:, c, ts(j, 512)],
                            start=(c == 0), stop=(c == IN_CHUNKS - 1),
                        )
                    vt = pro_vt.tile([128, 512], BF16, name="vt")
                    nc.scalar.activation(vt[:], ps[:], AF.Identity, bias=bv[:])
                    vps = pro_vps.tile([128, 512], BF16, name="vps")
                    for b in range(4):
                        nc.tensor.transpose(
                            vps[:, ts(b, 128)], vt[:, ts(b, 128)], ident_bf[:]
                        )
                    nc.vector.tensor_copy(v_sb[:, 4 * j : 4 * j + 4, :], vps[:])

            # ---------------- main loop over row tiles ----------------
            with (
                tc.tile_pool(name="mask_p", bufs=2) as mask_p,
                tc.tile_pool(name="em_p", bufs=2) as em_p,
                tc.tile_pool(name="stat_p", bufs=2) as stat_p,
                tc.tile_pool(name="pt_p", bufs=4) as pt_p,
                tc.tile_pool(name="co_p", bufs=2) as co_p,
                tc.tile_pool(name="qk_ps", bufs=3, space="PSUM") as qk_ps,
                tc.tile_pool(name="tp_ps", bufs=3, space="PSUM") as tp_ps,
                tc.tile_pool(name="pv_ps", bufs=2, space="PSUM") as pv_ps,
            ):
                for rt in [r for _ in range(main_reps) for r in range(RT)]:
                    mask_t = mask_p.tile([128, N_E], I32, name="mask_t")
                    nc.sync.dma_start(out=mask_t[:], in_=causal_mask[ts(rt, 128), :])

                    em = em_p.tile([128, N_E], F32, name="em")
                    for j in range(NE_CHUNKS):
                        ps = qk_ps.tile([128, 512], F32, name="ps")
                        nc.tensor.matmul(
                            ps[:], qt[:, ts(rt, 128)], kt[:, ts(j, 512)],
                            start=True, stop=True,
                        )
                        nc.scalar.activation(em[:, ts(j, 512)], ps[:], AF.Exp)

                    # em := mask * em (unnormalized P), rowsum comes free
                    rs = stat_p.tile([128, 1], F32, name="rs")
                    nc.vector.scalar_tensor_tensor(
                        out=em[:],
                        in0=mask_t[:],
                        scalar=1.0,
                        in1=em[:],
                        op0=ALU.mult,
                        op1=ALU.mult,
                        accum_out=rs[:],
                    )
                    recip = stat_p.tile([128, 1], F32, name="recip")
                    nc.vector.reciprocal(recip[:], rs[:])

                    # attn = em * recip, written into the dead mask buffer
                    attn_t = mask_t.bitcast(F32)
                    nc.scalar.mul(attn_t[:], em[:], recip[:])
                    nc.scalar.dma_start(out=attn_d[ts(rt, 128), :], in_=attn_t[:])

                    # transpose unnormalized P (f32 -> bf16 on batched copy-out),
                    # accumulate O' = P @ V ; O = O' * recip on the way out
                    co_ps = pv_ps.tile([128, D], F32, name="co_ps")
                    for g in range(NE_BLOCKS // 4):
                        tp = tp_ps.tile([128, 512], F32, name="tp")
                        for b in range(4):
                            blk = 4 * g + b
                            nc.tensor.transpose(
                                tp[:, ts(b, 128)], em[:, ts(blk, 128)], ident[:]
                            )
                        pt = pt_p.tile([128, 512], BF16, name="pt")
                        nc.vector.tensor_copy(pt[:], tp[:])
                        for b in range(4):
                            blk = 4 * g + b
                            nc.tensor.matmul(
                                co_ps[:], pt[:, ts(b, 128)], v_sb[:, blk, :],
                                start=(blk == 0), stop=(blk == NE_BLOCKS - 1),
                            )

                    co = co_p.tile([128, D], F32, name="co")
                    nc.scalar.mul(co[:], co_ps[:], recip[:])
                    nc.scalar.dma_start(out=causal_d[ts(rt, 128), :], in_=co[:])

    nc.compile()
    return nc


_NC_CACHE = {}


def _get_nc():
    if "nc" not in _NC_CACHE:
        _NC_CACHE["nc"] = build()
    return _NC_CACHE["nc"]


def kernel(emb_cause, emb_effect, causal_mask, Wq, bq, Wk, bk, Wv, bv):
    from concourse.bass_utils import run_bass_kernel_spmd

    emb_cause = np.ascontiguousarray(np.asarray(emb_cause, dtype=np.float32))
    emb_effect = np.ascontiguousarray(np.asarray(emb_effect, dtype=np.float32))
    causal_mask = np.ascontiguousarray(np.asarray(causal_mask, dtype=np.int32))
    shared = {
        "emb_effect": emb_effect,
        "Wq": np.ascontiguousarray(np.asarray(Wq, dtype=np.float32)),
        "Wk": np.ascontiguousarray(np.asarray(Wk, dtype=np.float32)),
        "Wv": np.ascontiguousarray(np.asarray(Wv, dtype=np.float32)),
        "bq": np.ascontiguousarray(np.asarray(bq, dtype=np.float32).reshape(D, 1)),
        "bk": np.ascontiguousarray(np.asarray(bk, dtype=np.float32).reshape(D, 1)),
        "bv": np.ascontiguousarray(np.asarray(bv, dtype=np.float32).reshape(D, 1)),
    }
    in_maps = []
    for c in range(N_CORES):
        rows = slice(c * RPC, (c + 1) * RPC)
        in_maps.append(
            {
                "emb_cause": np.ascontiguousarray(emb_cause[rows]),
                "causal_mask": np.ascontiguousarray(causal_mask[rows]),
                **shared,
            }
        )

    nc = _get_nc()
    res = run_bass_kernel_spmd(nc, in_maps, core_ids=list(range(N_CORES)))
    causal = np.concatenate([r["causal"] for r in res.results], axis=0)
    attn = np.concatenate([r["attn"] for r in res.results], axis=0)
    return causal, attn
